# revision 19
# baseline (speedup 1.0000x reference)
"""Causal attention (B=4, S=2048, D=1024, single head) on 8 trn2 NeuronCores.

Sharding: data-parallel over batch (4) x query-split (2) per batch, with
INTERLEAVED q-block pairing: global q-blocks (128 rows each) 0..15; core
(b, h) takes blocks {8g + 2j + h} for group g in {0,1}, position j in 0..3.
Adjacent pairing makes the SPMD-uniform causal work optimal: position (g,j)
accumulates AV over c = 8g + 2j + 2 k-blocks (total 72 visible-block units
per core vs 96/84 for contiguous grouping).

Each core projects K/V for its half of the keys (h=0: keys [0:1024],
h=1: [1024:2048]); pairwise AllGathers (K in fp8, V in bf16) share them.

Math notes (exactness-preserving):
  - softmax(q.(k0+bk)) == softmax(q.k0): bk dropped on device.
  - out = softmax(s)@(v0+bv) == softmax(s)@v0 + bv -> bv added on host.
  - max|score| ~ 2.7 -> exp without max-subtraction is safe.

Precision plan (rel-err budget 2e-2, measured ~1.5e-2):
  - Q/K/V projections in bf16 (fp32 PSUM accumulate).
  - q/k stored fp8e4 (values |q| <~ 3, well inside e4m3 range); the score
    matmul runs fp8 DoubleRow (2 contraction chunks per matmul, measured
    2.17x bf16 throughput on HW).
  - P = exp(SCALE*s) and V stay bf16; AV matmul bf16; output fp16.

Score panels are computed per-group [slot, 512 q] but column-trimmed:
slot s only needs q-columns of positions j with c_{g,j} > s, i.e. cols
[col0(s):512], eliminating all causal waste (72 block units of scores).
Masks: only 2 distinct [128,128] tiles per core (m0, m1) applied at slots
c-2, c-1 of each position: h=0: (tri, zero); h=1: (ones, tri).
"""

import numpy as np
import ml_dtypes

import concourse.bass as bass
import concourse.bacc as bacc
import concourse.mybir as mybir
import concourse.tile as tile
from concourse.bass_utils import run_bass_kernel_spmd

BF16 = mybir.dt.bfloat16
FP16 = mybir.dt.float16
FP8 = mybir.dt.float8e4
FP32 = mybir.dt.float32

B, S, D = 4, 2048, 1024
SCALE = 1.0 / np.sqrt(D)
P = 128                  # partition width
DC = D // P              # 8 feature/contraction chunks
DCH = DC // 2            # 4 DoubleRow chunk pairs
NQ = 1024                # query rows per core
GROUPS = 2
GQ = 512                 # queries per group
QC = GQ // P             # 4 positions (q-blocks) per group
KSLOTS = (8, 16)         # score slots per group (k-blocks 0..S_g-1)
# AV slot count per (group, position): c = 8g + 2j + 2
AV_CNT = ((2, 4, 6, 8), (10, 12, 14, 16))
# score column trim: slot s covers q-cols [COL0[g][s]:512]
COL0 = (
    [0, 0, 128, 128, 256, 256, 384, 384],
    [0] * 10 + [128, 128, 256, 256, 384, 384],
)

# Payload of the pairwise AllGathers (per core, send direction): K fp8 + V bf16.
COLLECTIVE_BYTES = DC * P * 1024 * 1 + 8 * P * 1024 * 2

_CACHE = {}


def _build_program(reps=1, loop_n=0):
    nc = bacc.Bacc("TRN2", target_bir_lowering=False, debug=False, num_devices=8)

    xTq = nc.dram_tensor("xTq", [P, DC * NQ], BF16, kind="ExternalInput").ap()
    xTk = nc.dram_tensor("xTk", [P, DC * 1024], BF16, kind="ExternalInput").ap()
    WqT = nc.dram_tensor("WqT", [P, DC * D], BF16, kind="ExternalInput").ap()
    WkT = nc.dram_tensor("WkT", [P, DC * D], BF16, kind="ExternalInput").ap()
    WvT = nc.dram_tensor("WvT", [P, DC * D], BF16, kind="ExternalInput").ap()
    bqv = nc.dram_tensor("bqv", [P, DC], FP32, kind="ExternalInput").ap()
    masks = nc.dram_tensor("masks", [P, 2 * P], BF16, kind="ExternalInput").ap()
    out = nc.dram_tensor("out", [NQ, D], FP16, kind="ExternalOutput").ap()

    with tile.TileContext(nc) as tc:
        if loop_n:
            with tc.For_i(0, loop_n, 1):
                _emit(tc, xTq, xTk, WqT, WkT, WvT, bqv, masks, out)
        else:
            for _ in range(reps):
                _emit(tc, xTq, xTk, WqT, WkT, WvT, bqv, masks, out)
    nc.compile()
    return nc


def _emit(tc, xTq, xTk, WqT, WkT, WvT, bqv, masks, out):
    nc = tc.nc

    with tc.tile_pool(name="persist", bufs=1) as persist, \
         tc.tile_pool(name="ktp", bufs=2) as ktp, \
         tc.tile_pool(name="dram", bufs=1, space="DRAM") as dram:
        # Persistent SBUF tensors.
        qt_sb = persist.tile([P, DC, NQ], FP8, name="qt_sb")
        kt_sb = ktp.tile([P, DC, S], FP8, tag="kt", name="kt_sb")
        v_sb = persist.tile([P, S // P, D], BF16, name="v_sb")
        bq_sb = persist.tile([P, DC], FP32, name="bq_sb")
        mk_sb = persist.tile([P, 2, P], BF16, name="mk_sb")
        ones_sb = persist.tile([P, 1], BF16, name="ones_sb")
        nc.sync.dma_start(bq_sb[:], bqv[:])
        nc.sync.dma_start(mk_sb[:], masks[:])
        nc.any.memset(ones_sb[:], 1.0)

        # DRAM bounce buffers for the pairwise K/V AllGathers.
        k_half = dram.tile([DC, P, 1024], FP8, name="k_half")
        v_half = dram.tile([8, P, 1024], BF16, name="v_half")
        k_full = dram.tile([2, DC, P, 1024], FP8, name="k_full")
        v_full = dram.tile([2, 8, P, 1024], BF16, name="v_full")

        # ---------------- Phase 1: projections ----------------
        # Order: K proj -> K AllGather (overlaps V proj) -> V proj ->
        # V AllGather (overlaps Q proj) -> Q proj -> reloads. xk/wk are
        # double-buffered so the next loop iteration's K path can load
        # while this iteration's attention phase runs.
        import os as _os
        no_coll = bool(_os.environ.get("NO_COLLECTIVE"))
        groups2 = [[0, 1], [2, 3], [4, 5], [6, 7]]
        with tc.tile_pool(name="wxk", bufs=2) as wxk, \
             tc.tile_pool(name="wx", bufs=1) as wx, \
             tc.tile_pool(name="stage", bufs=12) as stage, \
             tc.tile_pool(name="pj_psum", bufs=4, space="PSUM") as pj_psum:
            xk_sb = wxk.tile([P, DC, 1024], BF16, tag="xk", name="xk_sb")
            wk_sb = wxk.tile([P, DC, D], BF16, tag="wk", name="wk_sb")
            xq_sb = wx.tile([P, DC, NQ], BF16, name="xq_sb")
            wq_sb = wx.tile([P, DC, D], BF16, name="wq_sb")
            wv_sb = wx.tile([P, DC, D], BF16, name="wv_sb")
            # Batched loads in consumption order (K path first with the
            # first chunk split off so the dc-chain starts immediately);
            # big transfers keep SEQ DMA-dispatch count low, spread across
            # both HWDGE issuing engines for queue parallelism.
            for dc in range(DC):
                nc.sync.dma_start(xk_sb[:, dc, :], xTk[:, dc * 1024:(dc + 1) * 1024])
                nc.scalar.dma_start(wk_sb[:, dc, :], WkT[:, dc * D:(dc + 1) * D])
            for dc in range(DC):
                nc.sync.dma_start(wv_sb[:, dc, :], WvT[:, dc * D:(dc + 1) * D])
            for dc in range(DC):
                nc.scalar.dma_start(wq_sb[:, dc, :], WqT[:, dc * D:(dc + 1) * D])
            for dc in range(DC):
                nc.sync.dma_start(xq_sb[:, dc, :], xTq[:, dc * NQ:(dc + 1) * NQ])

            # K^T projection: KT[e, k] fp8 for own 1024 keys -> k_half.
            for ec in range(DC):
                for st in range(2):
                    ps = pj_psum.tile([P, 512], FP32, tag="pj", name="ps_k")
                    for dc in range(DC):
                        nc.tensor.matmul(
                            ps[:],
                            wk_sb[:, dc, ec * P:(ec + 1) * P],
                            xk_sb[:, dc, st * 512:(st + 1) * 512],
                            start=(dc == 0), stop=(dc == DC - 1),
                        )
                    sg = stage.tile([P, 512], FP8, tag="stage8", name="sg_k")
                    nc.vector.tensor_copy(sg[:], ps[:])
                    eng = nc.sync if (ec + st) % 2 == 0 else nc.scalar
                    eng.dma_start(k_half[ec, :, st * 512:(st + 1) * 512], sg[:])

            # K AllGather fires now; its transfer overlaps the V projection.
            if no_coll:
                nc.gpsimd.dma_start(k_full[0], k_half[:])
                nc.gpsimd.dma_start(k_full[1], k_half[:])
            else:
                nc.gpsimd.collective_compute(
                    "AllGather", mybir.AluOpType.bypass, replica_groups=groups2,
                    ins=[k_half.opt()], outs=[k_full.opt()],
                )
            # kt reload is queued early so it streams in as soon as the
            # gather lands, still overlapping V/Q projections.
            for r in range(2):
                for ec in range(DC):
                    nc.gpsimd.dma_start(
                        kt_sb[:, ec, r * 1024:(r + 1) * 1024], k_full[r, ec]
                    )

            # V projection: v[k, e] bf16 for own 1024 keys -> v_half.
            for kc in range(DC):
                for et in range(2):
                    ps = pj_psum.tile([P, 512], FP32, tag="pj", name="ps_v")
                    for dc in range(DC):
                        nc.tensor.matmul(
                            ps[:],
                            xk_sb[:, dc, kc * P:(kc + 1) * P],
                            wv_sb[:, dc, et * 512:(et + 1) * 512],
                            start=(dc == 0), stop=(dc == DC - 1),
                        )
                    sg = stage.tile([P, 512], BF16, tag="stage16", name="sg_v")
                    nc.vector.tensor_copy(sg[:], ps[:])
                    eng = nc.sync if (kc + et) % 2 == 0 else nc.scalar
                    eng.dma_start(v_half[kc, :, et * 512:(et + 1) * 512], sg[:])

            # V AllGather; its transfer overlaps the Q projection.
            if no_coll:
                nc.gpsimd.dma_start(v_full[0], v_half[:])
                nc.gpsimd.dma_start(v_full[1], v_half[:])
            else:
                nc.gpsimd.collective_compute(
                    "AllGather", mybir.AluOpType.bypass, replica_groups=groups2,
                    ins=[v_half.opt()], outs=[v_full.opt()],
                )
            for r in range(2):
                for kc in range(DC):
                    nc.gpsimd.dma_start(v_sb[:, r * DC + kc, :], v_full[r, kc])

            # Q^T projection: QT[e, q] fp8 (bias fused via ACT). st outer so
            # group 0's columns complete first and its scores can start.
            for st in range(2):
                for ec in range(DC):
                    ps = pj_psum.tile([P, 512], FP32, tag="pj", name="ps_q")
                    for dc in range(DC):
                        nc.tensor.matmul(
                            ps[:],
                            wq_sb[:, dc, ec * P:(ec + 1) * P],
                            xq_sb[:, dc, st * 512:(st + 1) * 512],
                            start=(dc == 0), stop=(dc == DC - 1),
                        )
                    nc.scalar.activation(
                        qt_sb[:, ec, st * 512:(st + 1) * 512],
                        ps[:],
                        mybir.ActivationFunctionType.Identity,
                        bias=bq_sb[:, ec:ec + 1],
                    )

        # ---------------- Phase 2: attention ----------------
        with tc.tile_pool(name="pt", bufs=KSLOTS[0] + KSLOTS[1] + 2) as pt_pool, \
             tc.tile_pool(name="ob", bufs=3) as ob_pool, \
             tc.tile_pool(name="sc_psum", bufs=3, space="PSUM") as sc_psum, \
             tc.tile_pool(name="av_psum", bufs=4, space="PSUM") as av_psum, \
             tc.tile_pool(name="dn_psum", bufs=1, space="PSUM") as dn_psum, \
             tc.tile_pool(name="sm", bufs=4) as sm_pool:

            pt_tiles = {}
            for g in range(GROUPS):
                for s in range(KSLOTS[g]):
                    c0 = COL0[g][s]
                    n = GQ - c0
                    # scores^T slot: [k(128) x q(n)] = KT_slot.T @ QT, fp8
                    # DoubleRow over 4 chunk pairs.
                    ps = sc_psum.tile([P, n], FP32, tag="sc", name="ps_sc")
                    for c in range(DCH):
                        nc.tensor.matmul(
                            ps[:],
                            kt_sb[:, 2 * c:2 * c + 2, s * P:(s + 1) * P],
                            qt_sb[:, 2 * c:2 * c + 2, g * GQ + c0:(g + 1) * GQ],
                            start=(c == 0), stop=(c == DCH - 1),
                            perf_mode=mybir.MatmulPerfMode.DoubleRow,
                        )
                    # P^T = exp(SCALE * scores^T) (bf16)
                    pt = pt_pool.tile([P, GQ], BF16, tag="pt", name="pt")
                    nc.scalar.activation(
                        pt[:, c0:GQ], ps[:], mybir.ActivationFunctionType.Exp,
                        scale=float(SCALE),
                    )
                    # mask slots c-2 (m0) and c-1 (m1) of each position:
                    # h=0: (tri, zero); h=1: (ones, tri).
                    if g == 0 or s >= 8:
                        j = (s - 8 * g) // 2
                        qs = j * P
                        nc.vector.tensor_tensor(
                            pt[:, qs:qs + P], pt[:, qs:qs + P], mk_sb[:, s % 2, :],
                            op=mybir.AluOpType.mult,
                        )
                    pt_tiles[(g, s)] = pt

            for g in range(GROUPS):
                dng = dn_psum.tile([P, QC], FP32, tag="dn", name="dng")
                for j in range(QC):
                    o0 = av_psum.tile([P, 512], FP32, tag="av", name="o0")
                    o1 = av_psum.tile([P, 512], FP32, tag="av", name="o1")
                    dn = dng[:, j:j + 1]
                    nslot = AV_CNT[g][j]
                    for s in range(nslot):
                        lhs = pt_tiles[(g, s)][:, j * P:(j + 1) * P]
                        st, sp = (s == 0), (s == nslot - 1)
                        nc.tensor.matmul(
                            o0[:], lhs, v_sb[:, s, 0:512], start=st, stop=sp
                        )
                        nc.tensor.matmul(
                            o1[:], lhs, v_sb[:, s, 512:1024], start=st, stop=sp
                        )
                        nc.tensor.matmul(
                            dn[:], lhs, ones_sb[:], start=st, stop=sp
                        )
                    inv = sm_pool.tile([P, 1], FP32, tag="inv", name="inv")
                    nc.vector.reciprocal(inv[:], dn[:])
                    ob = ob_pool.tile([P, D], FP16, tag="ob", name="ob")
                    row = g * GQ + j * P
                    nc.vector.tensor_scalar_mul(ob[:, 0:512], o0[:], inv[:])
                    nc.vector.tensor_scalar_mul(ob[:, 512:1024], o1[:], inv[:])
                    nc.sync.dma_start(out[row:row + P, :], ob[:])


def _chunked_T(a):
    """[rows, D] fp32 -> feature-major bf16 [P, DC*rows] (chunk-major free)."""
    rows = a.shape[0]
    t = np.ascontiguousarray(a.T)                      # [D, rows]
    t = t.reshape(DC, P, rows).transpose(1, 0, 2)      # [P, DC, rows]
    return np.ascontiguousarray(t.reshape(P, DC * rows)).astype(ml_dtypes.bfloat16)


def _q_blocks(half):
    """Global q-block index for core-local position (g, j), flattened."""
    return [8 * g + 2 * j + half for g in range(GROUPS) for j in range(QC)]


def _make_masks(half):
    """[P, 2*P] bf16: multiplicative masks applied at slots c-2, c-1."""
    tri = (np.arange(P)[:, None] <= np.arange(P)[None, :]).astype(np.float32)
    if half == 0:
        m = np.stack([tri, np.zeros((P, P), np.float32)], axis=1)
    else:
        m = np.stack([np.ones((P, P), np.float32), tri], axis=1)
    return np.ascontiguousarray(m.reshape(P, 2 * P)).astype(ml_dtypes.bfloat16)


def prepare_in_maps(x, Wq, bq, Wk, bk, Wv, bv):
    x = np.asarray(x, np.float32)
    masks_by_half = [_make_masks(0), _make_masks(1)]
    wqT = _chunked_T(np.asarray(Wq, np.float32))  # chunked(Wq^T) = [d part, e free]
    wkT = _chunked_T(np.asarray(Wk, np.float32))
    wvT = _chunked_T(np.asarray(Wv, np.float32))
    bq_t = np.ascontiguousarray(
        np.asarray(bq, np.float32).reshape(DC, P).T
    )  # [P, DC]

    in_maps = []
    for core in range(8):
        b, half = core // 2, core % 2
        qrows = np.concatenate(
            [np.arange(t * P, (t + 1) * P) for t in _q_blocks(half)]
        )
        krows = slice(0, 1024) if half == 0 else slice(1024, 2048)
        in_maps.append({
            "xTq": _chunked_T(x[b][qrows]),
            "xTk": _chunked_T(x[b][krows]),
            "WqT": wqT, "WkT": wkT, "WvT": wvT,
            "bqv": bq_t,
            "masks": masks_by_half[half],
        })
    return in_maps


def kernel(x, Wq, bq, Wk, bk, Wv, bv):
    in_maps = prepare_in_maps(x, Wq, bq, Wk, bk, Wv, bv)
    bv = np.asarray(bv, np.float32)

    import os
    reps = int(os.environ.get("BENCH_REPS", "1"))
    key = ("nc", reps)
    if key not in _CACHE:
        _CACHE[key] = _build_program(reps)
    res = run_bass_kernel_spmd(_CACHE[key], in_maps, list(range(8)))
    _CACHE["last_results"] = res

    out = np.empty((B, S, D), np.float32)
    for core in range(8):
        o = np.asarray(res.results[core]["out"]).astype(np.float32)
        b, half = core // 2, core % 2
        for i, t in enumerate(_q_blocks(half)):
            out[b, t * P:(t + 1) * P] = o[i * P:(i + 1) * P]
    out += bv
    return out


# revision 20
# speedup vs baseline: 1.0352x; 1.0352x over previous
"""Causal attention (B=4, S=2048, D=1024, single head) on 8 trn2 NeuronCores.

Sharding: data-parallel over batch (4) x query-split (2) per batch, with
INTERLEAVED q-block pairing: global q-blocks (128 rows each) 0..15; core
(b, h) takes blocks {8g + 2j + h} for group g in {0,1}, position j in 0..3.
Adjacent pairing makes the SPMD-uniform causal work optimal: position (g,j)
accumulates AV over c = 8g + 2j + 2 k-blocks (total 72 visible-block units
per core vs 96/84 for contiguous grouping).

Each core projects K/V for its half of the keys (h=0: keys [0:1024],
h=1: [1024:2048]); pairwise AllGathers (K in fp8, V in bf16) share them.

Math notes (exactness-preserving):
  - softmax(q.(k0+bk)) == softmax(q.k0): bk dropped on device.
  - out = softmax(s)@(v0+bv) == softmax(s)@v0 + bv -> bv added on host.
  - max|score| ~ 2.7 -> exp without max-subtraction is safe.

Precision plan (rel-err budget 2e-2, measured ~1.5e-2):
  - Q/K/V projections in bf16 (fp32 PSUM accumulate).
  - q/k stored fp8e4 (values |q| <~ 3, well inside e4m3 range); the score
    matmul runs fp8 DoubleRow (2 contraction chunks per matmul, measured
    2.17x bf16 throughput on HW).
  - P = exp(SCALE*s) and V stay bf16; AV matmul bf16; output fp16.

Score panels are computed per-group [slot, 512 q] but column-trimmed:
slot s only needs q-columns of positions j with c_{g,j} > s, i.e. cols
[col0(s):512], eliminating all causal waste (72 block units of scores).
Masks: only 2 distinct [128,128] tiles per core (m0, m1) applied at slots
c-2, c-1 of each position: h=0: (tri, zero); h=1: (ones, tri).
"""

import numpy as np
import ml_dtypes

import concourse.bass as bass
import concourse.bacc as bacc
import concourse.mybir as mybir
import concourse.tile as tile
from concourse.bass_utils import run_bass_kernel_spmd

BF16 = mybir.dt.bfloat16
FP16 = mybir.dt.float16
FP8 = mybir.dt.float8e4
FP32 = mybir.dt.float32

B, S, D = 4, 2048, 1024
SCALE = 1.0 / np.sqrt(D)
P = 128                  # partition width
DC = D // P              # 8 feature/contraction chunks
DCH = DC // 2            # 4 DoubleRow chunk pairs
NQ = 1024                # query rows per core
GROUPS = 2
GQ = 512                 # queries per group
QC = GQ // P             # 4 positions (q-blocks) per group
KSLOTS = (8, 16)         # score slots per group (k-blocks 0..S_g-1)
# AV slot count per (group, position): c = 8g + 2j + 2
AV_CNT = ((2, 4, 6, 8), (10, 12, 14, 16))
# score column trim: slot s covers q-cols [COL0[g][s]:512]
COL0 = (
    [0, 0, 128, 128, 256, 256, 384, 384],
    [0] * 10 + [128, 128, 256, 256, 384, 384],
)

# Payload of the pairwise AllGathers (per core, send direction): K fp8 + V bf16.
COLLECTIVE_BYTES = DC * P * 1024 * 1 + 8 * P * 1024 * 2

_CACHE = {}


def _build_program(reps=1, loop_n=0):
    nc = bacc.Bacc("TRN2", target_bir_lowering=False, debug=False, num_devices=8)

    xTq = nc.dram_tensor("xTq", [P, DC * NQ], BF16, kind="ExternalInput").ap()
    xTk = nc.dram_tensor("xTk", [P, DC * 1024], BF16, kind="ExternalInput").ap()
    WqT = nc.dram_tensor("WqT", [P, DC * D], BF16, kind="ExternalInput").ap()
    WkT = nc.dram_tensor("WkT", [P, DC * D], BF16, kind="ExternalInput").ap()
    WvT = nc.dram_tensor("WvT", [P, DC * D], BF16, kind="ExternalInput").ap()
    bqv = nc.dram_tensor("bqv", [P, DC], FP32, kind="ExternalInput").ap()
    masks = nc.dram_tensor("masks", [P, 2 * P], BF16, kind="ExternalInput").ap()
    out = nc.dram_tensor("out", [NQ, D], FP16, kind="ExternalOutput").ap()

    with tile.TileContext(nc) as tc:
        if loop_n:
            with tc.For_i(0, loop_n, 1):
                _emit(tc, xTq, xTk, WqT, WkT, WvT, bqv, masks, out)
        else:
            for _ in range(reps):
                _emit(tc, xTq, xTk, WqT, WkT, WvT, bqv, masks, out)
    nc.compile()
    return nc


def _emit(tc, xTq, xTk, WqT, WkT, WvT, bqv, masks, out):
    nc = tc.nc

    with tc.tile_pool(name="persist", bufs=1) as persist, \
         tc.tile_pool(name="dram", bufs=1, space="DRAM") as dram:
        # Persistent SBUF tensors.
        qt_sb = persist.tile([P, DC, NQ], FP8, name="qt_sb")
        kt_sb = persist.tile([P, DC, S], FP8, name="kt_sb")
        v_sb = persist.tile([P, S // P, D], BF16, name="v_sb")
        bq_sb = persist.tile([P, DC], FP32, name="bq_sb")
        mk_sb = persist.tile([P, 2, P], BF16, name="mk_sb")
        ones_sb = persist.tile([P, 1], BF16, name="ones_sb")
        nc.sync.dma_start(bq_sb[:], bqv[:])
        nc.sync.dma_start(mk_sb[:], masks[:])
        nc.any.memset(ones_sb[:], 1.0)

        # DRAM bounce buffers for the pairwise K/V AllGathers.
        k_half = dram.tile([DC, P, 1024], FP8, name="k_half")
        v_half = dram.tile([8, P, 1024], BF16, name="v_half")
        k_full = dram.tile([2, DC, P, 1024], FP8, name="k_full")
        v_full = dram.tile([2, 8, P, 1024], BF16, name="v_full")

        # ---------------- Phase 1: projections ----------------
        # Order: K proj -> K AllGather (overlaps V proj) -> V proj ->
        # V AllGather (overlaps Q proj) -> Q proj -> reloads. xk/wk are
        # double-buffered so the next loop iteration's K path can load
        # while this iteration's attention phase runs.
        import os as _os
        no_coll = bool(_os.environ.get("NO_COLLECTIVE"))
        groups2 = [[0, 1], [2, 3], [4, 5], [6, 7]]
        with tc.tile_pool(name="wxk", bufs=2) as wxk, \
             tc.tile_pool(name="wx", bufs=1) as wx, \
             tc.tile_pool(name="stage", bufs=12) as stage, \
             tc.tile_pool(name="pj_psum", bufs=4, space="PSUM") as pj_psum:
            xk_sb = wxk.tile([P, DC, 1024], BF16, tag="xk", name="xk_sb")
            wk_sb = wxk.tile([P, DC, D], BF16, tag="wk", name="wk_sb")
            xq_sb = wx.tile([P, DC, NQ], BF16, name="xq_sb")
            wq_sb = wx.tile([P, DC, D], BF16, name="wq_sb")
            wv_sb = wx.tile([P, DC, D], BF16, name="wv_sb")
            # Batched loads in consumption order (K path first with the
            # first chunk split off so the dc-chain starts immediately);
            # big transfers keep SEQ DMA-dispatch count low, spread across
            # both HWDGE issuing engines for queue parallelism.
            for dc in range(DC):
                nc.sync.dma_start(xk_sb[:, dc, :], xTk[:, dc * 1024:(dc + 1) * 1024])
                nc.scalar.dma_start(wk_sb[:, dc, :], WkT[:, dc * D:(dc + 1) * D])
            for dc in range(DC):
                nc.sync.dma_start(wv_sb[:, dc, :], WvT[:, dc * D:(dc + 1) * D])
            for dc in range(DC):
                nc.scalar.dma_start(wq_sb[:, dc, :], WqT[:, dc * D:(dc + 1) * D])
            for dc in range(DC):
                nc.sync.dma_start(xq_sb[:, dc, :], xTq[:, dc * NQ:(dc + 1) * NQ])

            # K^T projection: KT[e, k] fp8 for own 1024 keys -> k_half.
            for ec in range(DC):
                for st in range(2):
                    ps = pj_psum.tile([P, 512], FP32, tag="pj", name="ps_k")
                    for dc in range(DC):
                        nc.tensor.matmul(
                            ps[:],
                            wk_sb[:, dc, ec * P:(ec + 1) * P],
                            xk_sb[:, dc, st * 512:(st + 1) * 512],
                            start=(dc == 0), stop=(dc == DC - 1),
                        )
                    sg = stage.tile([P, 512], FP8, tag="stage8", name="sg_k")
                    nc.vector.tensor_copy(sg[:], ps[:])
                    eng = nc.sync if (ec + st) % 2 == 0 else nc.scalar
                    eng.dma_start(k_half[ec, :, st * 512:(st + 1) * 512], sg[:])

            # K AllGather fires now; its transfer overlaps the V projection.
            if no_coll:
                nc.gpsimd.dma_start(k_full[0], k_half[:])
                nc.gpsimd.dma_start(k_full[1], k_half[:])
            else:
                nc.gpsimd.collective_compute(
                    "AllGather", mybir.AluOpType.bypass, replica_groups=groups2,
                    ins=[k_half.opt()], outs=[k_full.opt()],
                )
            # kt reload is queued early so it streams in as soon as the
            # gather lands, still overlapping V/Q projections.
            for r in range(2):
                for ec in range(DC):
                    nc.gpsimd.dma_start(
                        kt_sb[:, ec, r * 1024:(r + 1) * 1024], k_full[r, ec]
                    )

            # V projection: v[k, e] bf16 for own 1024 keys -> v_half.
            for kc in range(DC):
                for et in range(2):
                    ps = pj_psum.tile([P, 512], FP32, tag="pj", name="ps_v")
                    for dc in range(DC):
                        nc.tensor.matmul(
                            ps[:],
                            xk_sb[:, dc, kc * P:(kc + 1) * P],
                            wv_sb[:, dc, et * 512:(et + 1) * 512],
                            start=(dc == 0), stop=(dc == DC - 1),
                        )
                    sg = stage.tile([P, 512], BF16, tag="stage16", name="sg_v")
                    nc.vector.tensor_copy(sg[:], ps[:])
                    eng = nc.sync if (kc + et) % 2 == 0 else nc.scalar
                    eng.dma_start(v_half[kc, :, et * 512:(et + 1) * 512], sg[:])

            # V AllGather; its transfer overlaps the Q projection.
            if no_coll:
                nc.gpsimd.dma_start(v_full[0], v_half[:])
                nc.gpsimd.dma_start(v_full[1], v_half[:])
            else:
                nc.gpsimd.collective_compute(
                    "AllGather", mybir.AluOpType.bypass, replica_groups=groups2,
                    ins=[v_half.opt()], outs=[v_full.opt()],
                )
            for r in range(2):
                for kc in range(DC):
                    nc.gpsimd.dma_start(v_sb[:, r * DC + kc, :], v_full[r, kc])

            # Q^T projection: QT[e, q] fp8 (bias fused via ACT). st outer so
            # group 0's columns complete first and its scores can start.
            for st in range(2):
                for ec in range(DC):
                    ps = pj_psum.tile([P, 512], FP32, tag="pj", name="ps_q")
                    for dc in range(DC):
                        nc.tensor.matmul(
                            ps[:],
                            wq_sb[:, dc, ec * P:(ec + 1) * P],
                            xq_sb[:, dc, st * 512:(st + 1) * 512],
                            start=(dc == 0), stop=(dc == DC - 1),
                        )
                    nc.scalar.activation(
                        qt_sb[:, ec, st * 512:(st + 1) * 512],
                        ps[:],
                        mybir.ActivationFunctionType.Identity,
                        bias=bq_sb[:, ec:ec + 1],
                    )

        # ---------------- Phase 2: attention ----------------
        with tc.tile_pool(name="pt", bufs=KSLOTS[0] + KSLOTS[1] + 2) as pt_pool, \
             tc.tile_pool(name="ob", bufs=3) as ob_pool, \
             tc.tile_pool(name="sc_psum", bufs=3, space="PSUM") as sc_psum, \
             tc.tile_pool(name="av_psum", bufs=4, space="PSUM") as av_psum, \
             tc.tile_pool(name="dn_psum", bufs=1, space="PSUM") as dn_psum, \
             tc.tile_pool(name="sm", bufs=4) as sm_pool:

            pt_tiles = {}
            for g in range(GROUPS):
                for s in range(KSLOTS[g]):
                    c0 = COL0[g][s]
                    n = GQ - c0
                    # scores^T slot: [k(128) x q(n)] = KT_slot.T @ QT, fp8
                    # DoubleRow over 4 chunk pairs.
                    ps = sc_psum.tile([P, n], FP32, tag="sc", name="ps_sc")
                    for c in range(DCH):
                        nc.tensor.matmul(
                            ps[:],
                            kt_sb[:, 2 * c:2 * c + 2, s * P:(s + 1) * P],
                            qt_sb[:, 2 * c:2 * c + 2, g * GQ + c0:(g + 1) * GQ],
                            start=(c == 0), stop=(c == DCH - 1),
                            perf_mode=mybir.MatmulPerfMode.DoubleRow,
                        )
                    # P^T = exp(SCALE * scores^T) (bf16)
                    pt = pt_pool.tile([P, GQ], BF16, tag="pt", name="pt")
                    nc.scalar.activation(
                        pt[:, c0:GQ], ps[:], mybir.ActivationFunctionType.Exp,
                        scale=float(SCALE),
                    )
                    # mask slots c-2 (m0) and c-1 (m1) of each position:
                    # h=0: (tri, zero); h=1: (ones, tri).
                    if g == 0 or s >= 8:
                        j = (s - 8 * g) // 2
                        qs = j * P
                        nc.vector.tensor_tensor(
                            pt[:, qs:qs + P], pt[:, qs:qs + P], mk_sb[:, s % 2, :],
                            op=mybir.AluOpType.mult,
                        )
                    pt_tiles[(g, s)] = pt

            for g in range(GROUPS):
                dng = dn_psum.tile([P, QC], FP32, tag="dn", name="dng")
                for j in range(QC):
                    o0 = av_psum.tile([P, 512], FP32, tag="av", name="o0")
                    o1 = av_psum.tile([P, 512], FP32, tag="av", name="o1")
                    dn = dng[:, j:j + 1]
                    nslot = AV_CNT[g][j]
                    for s in range(nslot):
                        lhs = pt_tiles[(g, s)][:, j * P:(j + 1) * P]
                        st, sp = (s == 0), (s == nslot - 1)
                        nc.tensor.matmul(
                            o0[:], lhs, v_sb[:, s, 0:512], start=st, stop=sp
                        )
                        nc.tensor.matmul(
                            o1[:], lhs, v_sb[:, s, 512:1024], start=st, stop=sp
                        )
                        nc.tensor.matmul(
                            dn[:], lhs, ones_sb[:], start=st, stop=sp
                        )
                    inv = sm_pool.tile([P, 1], FP32, tag="inv", name="inv")
                    nc.vector.reciprocal(inv[:], dn[:])
                    ob = ob_pool.tile([P, D], FP16, tag="ob", name="ob")
                    row = g * GQ + j * P
                    nc.vector.tensor_scalar_mul(ob[:, 0:512], o0[:], inv[:])
                    nc.vector.tensor_scalar_mul(ob[:, 512:1024], o1[:], inv[:])
                    nc.sync.dma_start(out[row:row + P, :], ob[:])


def _chunked_T(a):
    """[rows, D] fp32 -> feature-major bf16 [P, DC*rows] (chunk-major free)."""
    rows = a.shape[0]
    t = np.ascontiguousarray(a.T)                      # [D, rows]
    t = t.reshape(DC, P, rows).transpose(1, 0, 2)      # [P, DC, rows]
    return np.ascontiguousarray(t.reshape(P, DC * rows)).astype(ml_dtypes.bfloat16)


def _q_blocks(half):
    """Global q-block index for core-local position (g, j), flattened."""
    return [8 * g + 2 * j + half for g in range(GROUPS) for j in range(QC)]


def _make_masks(half):
    """[P, 2*P] bf16: multiplicative masks applied at slots c-2, c-1."""
    tri = (np.arange(P)[:, None] <= np.arange(P)[None, :]).astype(np.float32)
    if half == 0:
        m = np.stack([tri, np.zeros((P, P), np.float32)], axis=1)
    else:
        m = np.stack([np.ones((P, P), np.float32), tri], axis=1)
    return np.ascontiguousarray(m.reshape(P, 2 * P)).astype(ml_dtypes.bfloat16)


def prepare_in_maps(x, Wq, bq, Wk, bk, Wv, bv):
    x = np.asarray(x, np.float32)
    masks_by_half = [_make_masks(0), _make_masks(1)]
    wqT = _chunked_T(np.asarray(Wq, np.float32))  # chunked(Wq^T) = [d part, e free]
    wkT = _chunked_T(np.asarray(Wk, np.float32))
    wvT = _chunked_T(np.asarray(Wv, np.float32))
    bq_t = np.ascontiguousarray(
        np.asarray(bq, np.float32).reshape(DC, P).T
    )  # [P, DC]

    in_maps = []
    for core in range(8):
        b, half = core // 2, core % 2
        qrows = np.concatenate(
            [np.arange(t * P, (t + 1) * P) for t in _q_blocks(half)]
        )
        krows = slice(0, 1024) if half == 0 else slice(1024, 2048)
        in_maps.append({
            "xTq": _chunked_T(x[b][qrows]),
            "xTk": _chunked_T(x[b][krows]),
            "WqT": wqT, "WkT": wkT, "WvT": wvT,
            "bqv": bq_t,
            "masks": masks_by_half[half],
        })
    return in_maps


def kernel(x, Wq, bq, Wk, bk, Wv, bv):
    in_maps = prepare_in_maps(x, Wq, bq, Wk, bk, Wv, bv)
    bv = np.asarray(bv, np.float32)

    import os
    reps = int(os.environ.get("BENCH_REPS", "1"))
    key = ("nc", reps)
    if key not in _CACHE:
        _CACHE[key] = _build_program(reps)
    res = run_bass_kernel_spmd(_CACHE[key], in_maps, list(range(8)))
    _CACHE["last_results"] = res

    out = np.empty((B, S, D), np.float32)
    for core in range(8):
        o = np.asarray(res.results[core]["out"]).astype(np.float32)
        b, half = core // 2, core % 2
        for i, t in enumerate(_q_blocks(half)):
            out[b, t * P:(t + 1) * P] = o[i * P:(i + 1) * P]
    out += bv
    return out
